# revision 11
# baseline (speedup 1.0000x reference)
"""Trainium2 Bass kernel for nn_DistAttn (GNN edge-softmax message passing).

Strategy (8 NeuronCores, SPMD single program):
  - Destination-node sharding: core c owns dst nodes [5000c, 5000c+5000).
    Every edge lives on exactly one core (by dst), so the segmented softmax
    and the output rows are core-local -- no collectives needed.
  - Each core computes the full K = feat@Wk and h = feat@W_fc tables
    (replicated work), stores them interleaved as KH [N, 256] bf16 in DRAM,
    and Q = feat@Wq only for its own 5000-node range (SBUF-resident).
  - Edges are grouped by 128-node dst blocks. Per block, KH rows for the
    block's edges are fetched with dma_gather (512B rows, full DMA rate).
    dma_gather indices are int16, so the KH table is addressed as two
    base-offset slices (src < SPLIT and src >= SPLIT) with two gather calls.
  - Per 128-edge tile: a one-hot mask M2[j,d] = (dst_rel[j]==d) is built with
    one DVE tensor_scalar (iota vs per-partition dst_rel); PE transposes it
    to M1; Qg = M1^T @ Qwindow expands per-edge Q rows; a fused DVE
    scalar_tensor_tensor computes the per-edge dot e_j = sum_c Qg*Kg via
    accum_out. exp runs once per block on the ACT engine. U and denom
    accumulate in PSUM via matmuls with lhsT = M2*ex; the block epilogue
    normalizes U by denom and DMAs the 128 output rows.
  - All structure sizes (tiles per block) are computed from the actual
    indices at call time and padded to a uniform shape across cores so one
    compiled program serves all 8 cores.
"""

import os
import sys
import time

sys.path.insert(0, "/opt/trn_rl_repo")

import numpy as np

import concourse.bacc as bacc
import concourse.mybir as mybir
import concourse.tile as tile
from concourse import bass
from concourse.bass_utils import run_bass_kernel_spmd
from concourse.library_config import mlp as mlp_lib

dt = mybir.dt
BF16 = dt.np(dt.bfloat16)

N = 40000
E = 640000
F = 128
CORES = 8
NPC = N // CORES            # 5000 dst nodes per core
BLK = 128                   # dst nodes per block
NBLK = (NPC + BLK - 1) // BLK   # 40 blocks per core (last has 8 valid rows)
SPLIT = 24576               # low/high table split; N-SPLIT-1 < 2**15
NPADT = 40064               # node count padded to 128 multiple (313 tiles)
NT_GLOBAL = NPADT // 128    # 313
SCALE = float(np.sqrt(np.float32(F)))
PAD_DSTREL = 1000.0


def _host_prep(feat, W_fc, Wq, Wk, src, dst):
    """Shard edges by dst, group by (block, src-half), compute padded slot
    layout uniform across cores. Returns per-core input maps' index arrays
    plus T_low/T_high and the softmax shift c0."""
    core_of = dst // NPC
    d_loc = dst - core_of * NPC
    blk = d_loc // BLK
    half = (src >= SPLIT).astype(np.int64)

    # global group key: core, block, half
    gkey = (core_of * NBLK + blk) * 2 + half
    counts = np.bincount(gkey, minlength=CORES * NBLK * 2)
    low_counts = counts[0::2]
    high_counts = counts[1::2]
    T_low = int(np.ceil(low_counts.max() / 128))
    T_high = int(np.ceil(high_counts.max() / 128))
    T_blk = T_low + T_high
    ntiles = NBLK * T_blk

    order = np.argsort(gkey, kind="stable")
    gk_s = gkey[order]
    src_s = src[order]
    drel_s = (d_loc - blk * BLK)[order]
    half_s = half[order]

    starts = np.zeros(CORES * NBLK * 2 + 1, np.int64)
    np.cumsum(counts, out=starts[1:])
    pos = np.arange(E, dtype=np.int64) - starts[gk_s]

    # tile base (within core) for each group
    g_blk = (np.arange(CORES * NBLK * 2) // 2) % NBLK
    g_half = np.arange(CORES * NBLK * 2) % 2
    g_tile_base = g_blk * T_blk + np.where(g_half == 0, 0, T_low)

    slot = g_tile_base[gk_s] * 128 + pos          # slot within core
    lane = slot % 128
    tl = slot // 128                               # tile within core

    idx_val = np.where(half_s == 0, src_s, src_s - SPLIT).astype(np.int16)

    core_s = gk_s // (NBLK * 2)
    dstrel = np.full((CORES, 128, ntiles), PAD_DSTREL, np.float32)
    dstrel[core_s, lane, tl] = drel_s.astype(np.float32)

    ncols = ntiles * 8
    idx16 = np.zeros((CORES, 16, ncols), np.int16)
    col = g_tile_base[gk_s] * 8 + pos // 16
    row = pos % 16
    idx16[core_s, row, col] = idx_val
    idx16 = np.tile(idx16, (1, 8, 1))             # replicate to 128 partitions

    # softmax shift: any constant >= max(e) keeps exp in range
    Qh = feat @ Wq
    Kh = feat @ Wk
    emax = -np.inf
    for i in range(0, E, 131072):
        sl = slice(i, min(i + 131072, E))
        e = np.einsum("ij,ij->i", Qh[dst[sl]], Kh[src[sl]]) / SCALE
        emax = max(emax, float(e.max()))
    c0 = float(emax)

    return T_low, T_high, dstrel, idx16, c0


def _build_program(T_low, T_high, c0):
    STAGE = int(os.environ.get("K_STAGE", "9"))
    RUN_NBLK = int(os.environ.get("K_NBLK", str(NBLK)))
    T_blk = T_low + T_high
    ntiles = NBLK * T_blk
    ncols = ntiles * 8

    nc = bacc.Bacc("TRN2", target_bir_lowering=False, debug=False,
                   num_devices=CORES)

    featT_d = nc.dram_tensor("featT", [128, NPADT], dt.bfloat16,
                             kind="ExternalInput")
    featTq_d = nc.dram_tensor("featTq", [128, NBLK * 128], dt.bfloat16,
                              kind="ExternalInput")
    Wkh_d = nc.dram_tensor("Wkh", [128, 256], dt.bfloat16, kind="ExternalInput")
    Wq_d = nc.dram_tensor("Wq", [128, 128], dt.bfloat16, kind="ExternalInput")
    gidx_d = nc.dram_tensor("gidx", [128, ncols], dt.int16, kind="ExternalInput")
    dstrel_d = nc.dram_tensor("dstrel", [128, ntiles], dt.float32,
                              kind="ExternalInput")
    iota_d = nc.dram_tensor("iota", [128, 128], dt.bfloat16, kind="ExternalInput")
    ident_d = nc.dram_tensor("ident", [128, 128], dt.bfloat16,
                             kind="ExternalInput")
    ones_d = nc.dram_tensor("ones", [128, 1], dt.bfloat16, kind="ExternalInput")
    negc0_d = nc.dram_tensor("negc0", [128, 1], dt.float32, kind="ExternalInput")
    KH_d = nc.dram_tensor("KH", [NPADT, 256], dt.bfloat16)
    if os.environ.get("K_GSRC"):
        KH_g = nc.dram_tensor("gtab", [NPADT, 256], dt.bfloat16,
                              kind="ExternalInput")
    else:
        KH_g = KH_d
    rst_d = nc.dram_tensor("rst", [NPC, 128], dt.float32, kind="ExternalOutput")

    with tile.TileContext(nc) as tc:
        nc.gpsimd.load_library(mlp_lib)
        with tc.tile_pool(name="const", bufs=1) as cp:
            iota_sb = cp.tile([128, 128], dt.bfloat16, tag="iota")
            nc.sync.dma_start(out=iota_sb[:], in_=iota_d.ap())
            ident_sb = cp.tile([128, 128], dt.bfloat16, tag="ident")
            nc.sync.dma_start(out=ident_sb[:], in_=ident_d.ap())
            ones_sb = cp.tile([128, 1], dt.bfloat16, tag="ones")
            nc.sync.dma_start(out=ones_sb[:], in_=ones_d.ap())
            negc0_sb = cp.tile([128, 1], dt.float32, tag="negc0")
            nc.sync.dma_start(out=negc0_sb[:], in_=negc0_d.ap())
            Wkh_sb = cp.tile([128, 256], dt.bfloat16, tag="wkh")
            nc.sync.dma_start(out=Wkh_sb[:], in_=Wkh_d.ap())
            Wq_sb = cp.tile([128, 128], dt.bfloat16, tag="wq")
            nc.sync.dma_start(out=Wq_sb[:], in_=Wq_d.ap())
            gidx_sb = cp.tile([128, ncols], dt.int16, tag="gidx")
            nc.sync.dma_start(out=gidx_sb[:], in_=gidx_d.ap())
            dstrel_sb = cp.tile([128, ntiles], dt.float32, tag="dstrel")
            nc.sync.dma_start(out=dstrel_sb[:], in_=dstrel_d.ap())
            Q_sb = cp.tile([128, NBLK * 128], dt.bfloat16, tag="qsb")

            # ---- phase 1: node tables ----
            with tc.tile_pool(name="p1big", bufs=1) as p1big, \
                 tc.tile_pool(name="p1", bufs=3) as p1, \
                 tc.tile_pool(name="p1p", bufs=4, space="PSUM") as p1p:
                featT_sb = p1big.tile([128, NPADT], dt.bfloat16, tag="featT")
                nc.sync.dma_start(out=featT_sb[:], in_=featT_d.ap())
                featTq_sb = p1big.tile([128, NBLK * 128], dt.bfloat16,
                                       tag="featTq")
                nc.sync.dma_start(out=featTq_sb[:], in_=featTq_d.ap())
                for g in range(NT_GLOBAL):
                    ps = p1p.tile([128, 256], dt.float32, tag="khp")
                    nc.tensor.matmul(ps[:], lhsT=featT_sb[:, 128 * g:128 * (g + 1)],
                                     rhs=Wkh_sb[:], start=True, stop=True)
                    ev = p1.tile([128, 256], dt.bfloat16, tag="khe")
                    if g % 2 == 0:
                        nc.scalar.activation(ev[:], ps[:],
                                             mybir.ActivationFunctionType.Copy)
                    else:
                        nc.vector.tensor_copy(out=ev[:], in_=ps[:])
                    nc.sync.dma_start(out=KH_d.ap()[128 * g:128 * (g + 1), :],
                                      in_=ev[:])
                for b in range(NBLK):
                    ps = p1p.tile([128, 128], dt.float32, tag="qp")
                    nc.tensor.matmul(ps[:], lhsT=featTq_sb[:, 128 * b:128 * (b + 1)],
                                     rhs=Wq_sb[:], start=True, stop=True)
                    if b % 2 == 0:
                        nc.scalar.activation(Q_sb[:, 128 * b:128 * (b + 1)], ps[:],
                                             mybir.ActivationFunctionType.Copy)
                    else:
                        nc.vector.tensor_copy(out=Q_sb[:, 128 * b:128 * (b + 1)],
                                              in_=ps[:])

            # ---- phase 2: edges ----
            with tc.tile_pool(name="gbuf", bufs=2) as gp, \
                 tc.tile_pool(name="m2", bufs=2) as m2p, \
                 tc.tile_pool(name="sc", bufs=3) as scp, \
                 tc.tile_pool(name="st", bufs=2) as stp, \
                 tc.tile_pool(name="ps2", bufs=2, space="PSUM") as ps2, \
                 tc.tile_pool(name="psu", bufs=2, space="PSUM") as psu:
                for b in range(RUN_NBLK):
                    if STAGE < 2:
                        break
                    buf = gp.tile([128, T_blk, 256], dt.bfloat16, tag="gbuf")
                    cb = b * T_blk * 8
                    nc.gpsimd.dma_gather(
                        out_ap=buf[:, 0:T_low, :],
                        in_ap=KH_g.ap()[0:SPLIT, :],
                        idxs_ap=gidx_sb[:, cb:cb + T_low * 8],
                        num_idxs=T_low * 128, num_idxs_reg=T_low * 128,
                        elem_size=256, single_packet=False)
                    nc.gpsimd.dma_gather(
                        out_ap=buf[:, T_low:T_blk, :],
                        in_ap=KH_g.ap()[SPLIT:NPADT, :],
                        idxs_ap=gidx_sb[:, cb + T_low * 8:cb + T_blk * 8],
                        num_idxs=T_high * 128, num_idxs_reg=T_high * 128,
                        elem_size=256, single_packet=False)

                    e_strip = stp.tile([128, T_blk], dt.float32, tag="e")
                    ex_strip = stp.tile([128, T_blk], dt.float32, tag="ex")
                    m2s = []
                    for t in range(T_blk):
                        gt = b * T_blk + t
                        M2 = m2p.tile([128, 128], dt.bfloat16, tag=f"m2_{t}")
                        m2s.append(M2)
                        nc.vector.tensor_scalar(
                            out=M2[:], in0=iota_sb[:],
                            scalar1=dstrel_sb[:, gt:gt + 1], scalar2=None,
                            op0=mybir.AluOpType.is_equal)
                        if STAGE < 3:
                            continue
                        M1p = ps2.tile([128, 128], dt.bfloat16, space="PSUM",
                                       tag="m1p")
                        nc.tensor.transpose(M1p[:], M2[:], ident_sb[:])
                        M1 = scp.tile([128, 128], dt.bfloat16, tag="m1")
                        nc.scalar.activation(M1[:], M1p[:],
                                             mybir.ActivationFunctionType.Copy)
                        if STAGE < 4:
                            continue
                        Qg = ps2.tile([128, 128], dt.float32, space="PSUM",
                                      tag="qg")
                        nc.tensor.matmul(Qg[:], lhsT=M1[:],
                                         rhs=Q_sb[:, 128 * b:128 * (b + 1)],
                                         start=True, stop=True)
                        scr = scp.tile([128, 128], dt.bfloat16, tag="scr")
                        nc.vector.scalar_tensor_tensor(
                            out=scr[:], in0=Qg[:], scalar=1.0,
                            in1=buf[:, t, 0:128],
                            op0=mybir.AluOpType.mult, op1=mybir.AluOpType.mult,
                            accum_out=e_strip[:, t:t + 1])
                    if STAGE < 5:
                        continue
                    nc.scalar.activation(ex_strip[:], e_strip[:],
                                         mybir.ActivationFunctionType.Exp,
                                         bias=negc0_sb[:, 0:1], scale=1.0 / SCALE)
                    U = psu.tile([128, 128], dt.float32, space="PSUM", tag="U")
                    Dn = psu.tile([128, 8], dt.float32, space="PSUM", tag="Dn")
                    for t in range(T_blk):
                        M2x = scp.tile([128, 128], dt.bfloat16, tag="m2x")
                        nc.vector.tensor_scalar(
                            out=M2x[:], in0=m2s[t][:],
                            scalar1=ex_strip[:, t:t + 1], scalar2=None,
                            op0=mybir.AluOpType.mult)
                        nc.tensor.matmul(U[:, 0:128], lhsT=M2x[:],
                                         rhs=buf[:, t, 128:256],
                                         start=(t == 0), stop=(t == T_blk - 1))
                        nc.tensor.matmul(Dn[:, 0:1], lhsT=M2x[:],
                                         rhs=ones_sb[:],
                                         start=(t == 0), stop=(t == T_blk - 1))
                    dg = stp.tile([128, 1], dt.float32, tag="dg")
                    nc.vector.tensor_scalar(out=dg[:], in0=Dn[:, 0:1],
                                            scalar1=1e-30, scalar2=None,
                                            op0=mybir.AluOpType.add)
                    rr = stp.tile([128, 1], dt.float32, tag="rr")
                    nc.vector.reciprocal(rr[:], dg[:])
                    ro = scp.tile([128, 128], dt.float32, tag="ro")
                    nc.vector.tensor_scalar(out=ro[:], in0=U[:, 0:128],
                                            scalar1=rr[:, 0:1], scalar2=None,
                                            op0=mybir.AluOpType.mult)
                    rows = min(BLK, NPC - b * BLK)
                    nc.sync.dma_start(out=rst_d.ap()[b * BLK:b * BLK + rows, :],
                                      in_=ro[:rows, :])
    nc.finalize()
    return nc


_CACHE = {}


def kernel(feat, loc, W_fc, Wq, Wk, Wq2, Wk2, G_w, embed, boundaries,
           src, dst, inter_ids, **_ignored):
    feat = np.asarray(feat, np.float32)
    W_fc = np.asarray(W_fc, np.float32)
    Wq = np.asarray(Wq, np.float32)
    Wk = np.asarray(Wk, np.float32)
    src = np.asarray(src).astype(np.int64)
    dst = np.asarray(dst).astype(np.int64)

    T_low, T_high, dstrel, idx16, c0 = _host_prep(feat, W_fc, Wq, Wk, src, dst)

    key = (T_low, T_high, round(c0, 4))
    if key not in _CACHE:
        _CACHE[key] = _build_program(T_low, T_high, c0)
    nc = _CACHE[key]

    featT_pad = np.zeros((128, NPADT), BF16)
    featT_pad[:, :N] = feat.T.astype(BF16)
    Wkh = np.concatenate([Wk, W_fc], axis=1).astype(BF16)
    Wq_b = Wq.astype(BF16)
    iota = np.broadcast_to(np.arange(128, dtype=np.float32), (128, 128))
    iota = np.ascontiguousarray(iota).astype(BF16)
    ident = np.eye(128, dtype=np.float32).astype(BF16)
    ones = np.ones((128, 1), np.float32).astype(BF16)

    in_maps = []
    for c in range(CORES):
        fq = np.zeros((128, NBLK * 128), BF16)
        lo = c * NPC
        hi = min(lo + NBLK * 128, N)
        fq[:, :hi - lo] = featT_pad[:, lo:hi]
        in_maps.append({
            "featT": featT_pad,
            "featTq": fq,
            "Wkh": Wkh,
            "Wq": Wq_b,
            "gidx": np.ascontiguousarray(idx16[c]),
            "dstrel": np.ascontiguousarray(dstrel[c]),
            "iota": iota,
            "ident": ident,
            "ones": ones,
            "negc0": np.full((128, 1), -c0, np.float32),
            **({"gtab": np.zeros((NPADT, 256), BF16)} if os.environ.get("K_GSRC") else {}),
        })

    res = run_bass_kernel_spmd(nc, in_maps, core_ids=list(range(CORES)))
    out = np.concatenate([res.results[c]["rst"] for c in range(CORES)], axis=0)
    return out.astype(np.float32)


if __name__ == "__main__":
    rng = np.random.default_rng(0)
    feat = rng.standard_normal((N, F), dtype=np.float32)
    W = {k: (rng.standard_normal((F, F), dtype=np.float32) * 0.09)
         for k in ("W_fc", "Wq", "Wk")}
    src = rng.integers(0, N, E)
    dst = rng.integers(0, N, E)
    t0 = time.time()
    out = kernel(feat=feat, loc=None, W_fc=W["W_fc"], Wq=W["Wq"], Wk=W["Wk"],
                 Wq2=None, Wk2=None, G_w=None, embed=None, boundaries=None,
                 src=src, dst=dst, inter_ids=None)
    print("kernel done", time.time() - t0, out.shape, out.dtype)
